# revision 9
# baseline (speedup 1.0000x reference)
"""HONU order-3 kernel for 8 TRN2 NeuronCores.

Math: out[b] = sum_{i<=j<=k} w_ijk * xf_i * xf_j * xf_k,  xf = [1, x] (127 feats).

Restructuring: group combos by pair (i,j) (lex order => per-pair weights are a
contiguous slice of `weights`).  Let W[(i,j), k] = w_ijk for k>=j (0 otherwise).
Then  Z[b,(i,j)] = sum_k W[(i,j),k] * xf[b,k]   (a dense matmul), and
      out[b]     = sum_{(i,j)} Q[b,(i,j)] * Z[b,(i,j)],   Q[b,(i,j)] = xf_i*xf_j.

Sharding: pair-rows i are dealt round-robin to the 8 cores (core c gets rows
i = 8t + c, t = 0..15); class t occupies columns [OFFS[t], OFFS[t+1]) covering
j in [8t, 128) (8-aligned; leading j in [8t,i) and j=127 carry zero weights).
NCOLS = 1088 per core.

The pair-products Q are BUILT ON THE HOST (they are pure input data) and
shipped as bf16, so the only on-chip epilogue work is ONE fused
multiply+accumulate (scalar_tensor_tensor) per 128-batch tile, reading Z
straight from PSUM.  Matmuls run in bf16 (total rel err ~1.3e-3, tolerance
2e-2).  res [128,2] is PE-transposed to [2,128] so the output DMA is 2
contiguous 512B descriptors.

x is replicated; each core returns a [2,128] partial that the host sums.
"""

import numpy as np
import ml_dtypes

import concourse.bass as bass
import concourse.bacc as bacc
import concourse.tile as tile
import concourse.mybir as mybir
from concourse.bass_utils import run_bass_kernel_spmd
from concourse.masks import make_identity

F32 = mybir.dt.float32
BF16 = mybir.dt.bfloat16
BF16_NP = ml_dtypes.bfloat16

P = 128
NF = 127            # features incl. bias
B = 256             # batch
NCLASS = 16
WIDTHS = [128 - 8 * t for t in range(NCLASS)]
OFFS = np.concatenate([[0], np.cumsum(WIDTHS)])
NCOLS = int(OFFS[-1])                                   # 1088
CHUNKS = [(0, 512), (512, 1024), (1024, NCOLS)]         # matmul N <= 512

_CACHE = {}


def _build_nc():
    nc = bacc.Bacc("TRN2", target_bir_lowering=False, debug=False)
    xt = nc.dram_tensor("xt", [P, B], BF16, kind="ExternalInput")   # xf^T padded
    qhs = [nc.dram_tensor(f"qh{bt}", [P, NCOLS], BF16, kind="ExternalInput")
           for bt in range(2)]
    wds = [nc.dram_tensor(f"wd{ci}", [P, hi - lo], BF16, kind="ExternalInput")
           for ci, (lo, hi) in enumerate(CHUNKS)]
    out = nc.dram_tensor("out", [2, P], F32, kind="ExternalOutput")

    with tile.TileContext(nc) as tc:
        with (
            tc.tile_pool(name="const", bufs=1) as cpool,
            tc.tile_pool(name="ps", bufs=2, space="PSUM") as ps,
            tc.tile_pool(name="pst", bufs=1, space="PSUM") as pst,
        ):
            # q0 first (gates dot-t0); weights+xt spread over the queues
            qh_t = [cpool.tile([P, NCOLS], BF16, tag=f"qh{bt}", name=f"qh{bt}_t")
                    for bt in range(2)]
            wd_t = [cpool.tile([P, hi - lo], BF16, tag=f"wd{ci}", name=f"wd{ci}_t")
                    for ci, (lo, hi) in enumerate(CHUNKS)]
            xt_t = cpool.tile([P, B], BF16, tag="xt")
            nc.gpsimd.dma_start(qh_t[0][:], qhs[0][:])
            nc.sync.dma_start(wd_t[0][:], wds[0][:])
            nc.sync.dma_start(wd_t[1][:], wds[1][:])
            nc.sync.dma_start(wd_t[2][:], wds[2][:])
            nc.scalar.dma_start(xt_t[:], xt[:])
            nc.scalar.dma_start(qh_t[1][:], qhs[1][:])

            ident = cpool.tile([P, P], F32, tag="ident")
            make_identity(nc, ident[:])

            res = cpool.tile([P, 2], F32, tag="res")
            e = cpool.tile([P, NCOLS], F32, tag="e")
            for bt in range(2):
                z_ps = ps.tile([P, NCOLS], F32, tag="z", name=f"z{bt}_ps")
                for ci, (lo, hi) in enumerate(CHUNKS):
                    nc.tensor.matmul(
                        z_ps[:, lo:hi],
                        xt_t[:, bt * P:(bt + 1) * P], wd_t[ci][:],
                        start=True, stop=True,
                    )
                # fused multiply+reduce over all 1088 cols straight from PSUM
                nc.vector.scalar_tensor_tensor(
                    out=e[:],
                    in0=z_ps[:],
                    scalar=1.0,
                    in1=qh_t[bt][:],
                    op0=mybir.AluOpType.mult,
                    op1=mybir.AluOpType.mult,
                    accum_out=res[:, bt:bt + 1],
                )
            # [128,2] -> [2,128] so the out DMA is 2 contiguous descriptors
            tps = pst.tile([2, P], F32, tag="tps")
            nc.tensor.transpose(tps[:], res[:], ident[:])
            osb = cpool.tile([2, P], F32, tag="osb")
            nc.vector.tensor_copy(osb[:], tps[:])
            nc.scalar.dma_start(out[:], osb[:])
    nc.compile()
    return nc


def _prep_inputs(x, weights, comb_idx):
    """Host-side layout prep: xf paddings, pair-products Q, dense weight chunks."""
    x = np.ascontiguousarray(np.asarray(x, dtype=np.float32))
    w = np.asarray(weights, dtype=np.float32).ravel()
    ci = np.asarray(comb_idx)
    i_, j_ = ci[:, 0].astype(np.int64), ci[:, 1].astype(np.int64)
    k_ = ci[:, 2].astype(np.int64)

    xf = np.concatenate([np.ones((B, 1), np.float32), x], axis=1)   # [256,127]
    xbp = np.zeros((B, P), np.float32)
    xbp[:, :NF] = xf

    xt = np.zeros((P, B), np.float32)
    xt[:NF, :] = xf.T
    xt16 = xt.astype(BF16_NP)

    # lex pair-row index of each combo
    ar = np.arange(NF, dtype=np.int64)
    rsp = ar * NF - (ar * (ar - 1)) // 2
    q = rsp[i_] + (j_ - i_)
    Wd = np.zeros((8128, NF), np.float32)
    Wd[q, k_] = w

    in_maps = []
    for c in range(8):
        big = np.zeros((P, NCOLS), np.float32)
        Q = np.zeros((B, NCOLS), np.float32)
        for t in range(NCLASS):
            i = 8 * t + c
            if i > 126:
                continue
            o = int(OFFS[t])
            Q[:, o:o + WIDTHS[t]] = xf[:, i:i + 1] * xbp[:, 8 * t:P]
            p0 = int(rsp[i])
            big[:NF, o + (i - 8 * t): o + (NF - 8 * t)] = Wd[p0:p0 + (NF - i)].T
        big16 = big.astype(BF16_NP)
        Q16 = Q.astype(BF16_NP)
        m = {"xt": xt16}
        for bt in range(2):
            m[f"qh{bt}"] = np.ascontiguousarray(Q16[bt * P:(bt + 1) * P])
        for ci2, (lo, hi) in enumerate(CHUNKS):
            m[f"wd{ci2}"] = np.ascontiguousarray(big16[:, lo:hi])
        in_maps.append(m)
    return in_maps


def _get_nc():
    if "nc" not in _CACHE:
        _CACHE["nc"] = _build_nc()
    return _CACHE["nc"]


def run_spmd(x, weights, comb_idx, trace=False):
    nc = _get_nc()
    in_maps = _prep_inputs(x, weights, comb_idx)
    res = run_bass_kernel_spmd(nc, in_maps, list(range(8)), trace=trace)
    acc = np.zeros((2, P), np.float64)
    for c in range(8):
        acc += res.results[c]["out"].astype(np.float64)
    return acc.reshape(B, 1).astype(np.float32), res


def kernel(x, weights, comb_idx):
    out, _ = run_spmd(x, weights, comb_idx, trace=False)
    return out


# revision 11
# speedup vs baseline: 1.0720x; 1.0720x over previous
"""HONU order-3 kernel for 8 TRN2 NeuronCores.

Math: out[b] = sum_{i<=j<=k} w_ijk * xf_i * xf_j * xf_k,  xf = [1, x] (127 feats).

Restructuring: group combos by pair (i,j) (lex order => per-pair weights are a
contiguous slice of `weights`).  Let W[(i,j), k] = w_ijk for k>=j (0 otherwise).
Then  Z[b,(i,j)] = sum_k W[(i,j),k] * xf[b,k]   (a dense matmul), and
      out[b]     = sum_{(i,j)} Q[b,(i,j)] * Z[b,(i,j)],   Q[b,(i,j)] = xf_i*xf_j.

Sharding: pair-rows i are dealt round-robin to the 8 cores (core c gets rows
i = 8t + c, t = 0..15); class t occupies columns [OFFS[t], OFFS[t+1]) covering
j in [8t, 128) (8-aligned; leading j in [8t,i) and j=127 carry zero weights).
NCOLS = 1088 per core.

The pair-products Q are BUILT ON THE HOST (they are pure input data) and
shipped as bf16, so the only on-chip epilogue work is ONE fused
multiply+accumulate (scalar_tensor_tensor) per 128-batch tile, reading Z
straight from PSUM.  Matmuls run in bf16 (total rel err ~1.3e-3, tolerance
2e-2).  res [128,2] is PE-transposed to [2,128] so the output DMA is 2
contiguous 512B descriptors.

x is replicated; each core returns a [2,128] partial that the host sums.
"""

import numpy as np
import ml_dtypes

import concourse.bass as bass
import concourse.bacc as bacc
import concourse.tile as tile
import concourse.mybir as mybir
from concourse.bass_utils import run_bass_kernel_spmd
from concourse.masks import make_identity

F32 = mybir.dt.float32
BF16 = mybir.dt.bfloat16
BF16_NP = ml_dtypes.bfloat16

P = 128
NF = 127            # features incl. bias
B = 256             # batch
NCLASS = 16
WIDTHS = [128 - 8 * t for t in range(NCLASS)]
OFFS = np.concatenate([[0], np.cumsum(WIDTHS)])
NCOLS = int(OFFS[-1])                                   # 1088
CHUNKS = [(0, 512), (512, 1024), (1024, NCOLS)]         # matmul N <= 512

_CACHE = {}


def _build_nc():
    nc = bacc.Bacc("TRN2", target_bir_lowering=False, debug=False)
    xt = nc.dram_tensor("xt", [P, B], BF16, kind="ExternalInput")   # xf^T padded
    qhs = [nc.dram_tensor(f"qh{bt}", [P, NCOLS], BF16, kind="ExternalInput")
           for bt in range(2)]
    wd = nc.dram_tensor("wd", [P, NCOLS], BF16, kind="ExternalInput")
    out = nc.dram_tensor("out", [2, P], F32, kind="ExternalOutput")

    with tile.TileContext(nc) as tc:
        with (
            tc.tile_pool(name="const", bufs=1) as cpool,
            tc.tile_pool(name="ps", bufs=2, space="PSUM") as ps,
            tc.tile_pool(name="pst", bufs=1, space="PSUM") as pst,
        ):
            # q0 first (gates dot-t0); weights+xt spread over the queues
            qh_t = [cpool.tile([P, NCOLS], BF16, tag=f"qh{bt}", name=f"qh{bt}_t")
                    for bt in range(2)]
            wd_t = cpool.tile([P, NCOLS], BF16, tag="wd")
            xt_t = cpool.tile([P, B], BF16, tag="xt")
            nc.gpsimd.dma_start(qh_t[0][:], qhs[0][:])
            nc.sync.dma_start(wd_t[:], wd[:])
            nc.scalar.dma_start(xt_t[:], xt[:])
            nc.scalar.dma_start(qh_t[1][:], qhs[1][:])

            ident = cpool.tile([P, P], F32, tag="ident")
            make_identity(nc, ident[:])

            res = cpool.tile([P, 2], F32, tag="res")
            e = cpool.tile([P, NCOLS], F32, tag="e")
            for bt in range(2):
                z_ps = ps.tile([P, NCOLS], F32, tag="z", name=f"z{bt}_ps")
                for lo, hi in CHUNKS:
                    nc.tensor.matmul(
                        z_ps[:, lo:hi],
                        xt_t[:, bt * P:(bt + 1) * P], wd_t[:, lo:hi],
                        start=True, stop=True,
                    )
                # fused multiply+reduce over all 1088 cols straight from PSUM
                nc.vector.scalar_tensor_tensor(
                    out=e[:],
                    in0=z_ps[:],
                    scalar=1.0,
                    in1=qh_t[bt][:],
                    op0=mybir.AluOpType.mult,
                    op1=mybir.AluOpType.mult,
                    accum_out=res[:, bt:bt + 1],
                )
            # [128,2] -> [2,128] so the out DMA is 2 contiguous descriptors
            tps = pst.tile([2, P], F32, tag="tps")
            nc.tensor.transpose(tps[:], res[:], ident[:])
            osb = cpool.tile([2, P], F32, tag="osb")
            nc.vector.tensor_copy(osb[:], tps[:])
            nc.scalar.dma_start(out[:], osb[:])
    nc.compile()
    return nc


def _prep_inputs(x, weights, comb_idx):
    """Host-side layout prep: xf paddings, pair-products Q, dense weight chunks."""
    x = np.ascontiguousarray(np.asarray(x, dtype=np.float32))
    w = np.asarray(weights, dtype=np.float32).ravel()
    ci = np.asarray(comb_idx)
    i_, j_ = ci[:, 0].astype(np.int64), ci[:, 1].astype(np.int64)
    k_ = ci[:, 2].astype(np.int64)

    xf = np.concatenate([np.ones((B, 1), np.float32), x], axis=1)   # [256,127]
    xbp = np.zeros((B, P), np.float32)
    xbp[:, :NF] = xf

    xt = np.zeros((P, B), np.float32)
    xt[:NF, :] = xf.T
    xt16 = xt.astype(BF16_NP)

    # lex pair-row index of each combo
    ar = np.arange(NF, dtype=np.int64)
    rsp = ar * NF - (ar * (ar - 1)) // 2
    q = rsp[i_] + (j_ - i_)
    Wd = np.zeros((8128, NF), np.float32)
    Wd[q, k_] = w

    in_maps = []
    for c in range(8):
        big = np.zeros((P, NCOLS), np.float32)
        Q = np.zeros((B, NCOLS), np.float32)
        for t in range(NCLASS):
            i = 8 * t + c
            if i > 126:
                continue
            o = int(OFFS[t])
            Q[:, o:o + WIDTHS[t]] = xf[:, i:i + 1] * xbp[:, 8 * t:P]
            p0 = int(rsp[i])
            big[:NF, o + (i - 8 * t): o + (NF - 8 * t)] = Wd[p0:p0 + (NF - i)].T
        big16 = big.astype(BF16_NP)
        Q16 = Q.astype(BF16_NP)
        m = {"xt": xt16, "wd": big16}
        for bt in range(2):
            m[f"qh{bt}"] = np.ascontiguousarray(Q16[bt * P:(bt + 1) * P])
        in_maps.append(m)
    return in_maps


def _get_nc():
    if "nc" not in _CACHE:
        _CACHE["nc"] = _build_nc()
    return _CACHE["nc"]


def run_spmd(x, weights, comb_idx, trace=False):
    nc = _get_nc()
    in_maps = _prep_inputs(x, weights, comb_idx)
    res = run_bass_kernel_spmd(nc, in_maps, list(range(8)), trace=trace)
    acc = np.zeros((2, P), np.float64)
    for c in range(8):
        acc += res.results[c]["out"].astype(np.float64)
    return acc.reshape(B, 1).astype(np.float32), res


def kernel(x, weights, comb_idx):
    out, _ = run_spmd(x, weights, comb_idx, trace=False)
    return out
